# revision 28
# baseline (speedup 1.0000x reference)
"""Trainium2 Bass kernel for nn_AttentionBlock (B=32, C=256, H=W=32).

v5: all matmuls in fp8e4 DoubleRow perf mode (K=256 per instruction,
2x PE throughput vs bf16).

Computation (per batch element b), algebraically restructured:
    scores^T = x^T (M x)        with M = Wk^T Wq  [bilinear form]
    E        = exp(scores^T / 16 - 3)   # -3: softmax-invariant shift so
                                        # E fits fp8e4 (TRN max normal 240)
    Z[hw]    = sum_HW E[HW, hw]         # fp8 ones DoubleRow matmuls
    U^T      = x^T (w_out w_v)^T        # w_out folded into V
    out      = (U^T-contract E) * (1/Z) + x

Precision: all five matmul groups (G, U, S, Z, F) run fp8e4 DoubleRow
with fp32 PSUM accumulation; residual path and output store are bf16.
CPU simulation of this exact quantization chain: rel err ~5.6e-3
(tolerance 2e-2).

Sharding: data-parallel over batch, 4 batch elements per core, 8 cores,
weights replicated.

Pipeline (window W_b ~ one batch, all deferred work interleaved into
the fused pass so the PE never waits on the slower ACT exp):
  PE   : 8 fused steps [U(b) + S(b) n0/n1] with Z(b-1) at steps 1-2,
         G(b+1) at steps 2,4,6,7, F(b-1) spread over steps 3-8
  ACT  : exp over score pairs [128,1024] (2-bank PSUM tiles), fp8 out
  GPSIMD: even U evicts, finalize muls (F read from PSUM x rb -> bf16)
  DVE  : odd U evicts, G evicts, 1/Z recips, residual adds (bf16 2x)
  Sync : x8/xb loads, rb DRAM broadcast, bf16 output stores
PSUM (8 banks): sps scores 2x[128,2,512] (4) + fps F/Z-ring 2x[128,512]
(2) + gps G (1) + ups U (1).

Env knobs:
    ATTN_TRACE   = 0 | 1   (collect NTFF trace via run_bass_kernel_spmd)
"""

import math
import os
import sys

import numpy as np

B, C, HW = 32, 256, 1024
NCORES = 8
BPC = B // NCORES  # batch elements per core
CH_T = C // 128  # channel partition tiles (2)
HW_T = HW // 128  # spatial partition tiles (8)
NF = 512  # matmul free-dim chunk (one PSUM bank of fp32)
N_CH = HW // NF  # free-dim chunks (2)
ESHIFT = 3.0  # exp(s/16 - ESHIFT): keeps E in fp8e4 range

_cache = {}
last_results = None  # BassKernelResults of the most recent run (for test.py)


def _ensure_path():
    for p in ("/opt/trn_rl_repo",):
        if os.path.isdir(p) and p not in sys.path:
            sys.path.append(p)


def _build(zero_bias: bool):
    """Build + compile the Bass kernel once. Returns the compiled nc."""
    _ensure_path()
    import concourse.bass as bass
    import concourse.mybir as mybir
    import concourse.tile as tile
    from concourse import bacc

    f32 = mybir.dt.float32
    bf16 = mybir.dt.bfloat16
    fp8 = mybir.dt.float8e4
    Act = mybir.ActivationFunctionType
    Alu = mybir.AluOpType
    DR = mybir.MatmulPerfMode.DoubleRow

    nc = bacc.Bacc(
        "TRN2", target_bir_lowering=False, debug=False, enable_asserts=False
    )

    x8_d = nc.dram_tensor("x8", [BPC, C, HW], fp8, kind="ExternalInput")
    xb_d = nc.dram_tensor("xb", [BPC, C, HW], bf16, kind="ExternalInput")
    mT8_d = nc.dram_tensor("mT8", [C, C], fp8, kind="ExternalInput")
    wu8_d = nc.dram_tensor("wu8", [C, C], fp8, kind="ExternalInput")
    bias_d = nc.dram_tensor("bias", [128, 4], f32, kind="ExternalInput")
    out_d = nc.dram_tensor("out", [BPC, C, HW], bf16, kind="ExternalOutput")

    def ns(n):
        return slice(n * NF, (n + 1) * NF)

    def mb(m):
        return slice(m * 128, (m + 1) * 128)

    with tile.TileContext(nc) as tc:
        with (
            tc.tile_pool(name="const", bufs=1) as const,
            tc.tile_pool(name="xp", bufs=3) as xp,
            tc.tile_pool(name="xbp", bufs=2) as xbp,
            tc.tile_pool(name="gp", bufs=2) as gp,
            tc.tile_pool(name="up", bufs=2) as up,
            tc.tile_pool(name="ep", bufs=2) as ep,
            tc.tile_pool(name="rp", bufs=2) as rp,
            tc.tile_pool(name="tp", bufs=4) as tp,
            tc.tile_pool(name="op", bufs=6) as op_,
            tc.tile_pool(name="ps", bufs=1, space="PSUM") as ps,
            tc.tile_pool(name="rd", bufs=2, space="DRAM") as rd,
        ):
            # PSUM tags (8 banks):
            #   sps [128,2,512] bufs=2 -> 4 banks (scores; exp N=1024)
            #   fps [128,512]   bufs=2 -> 2 banks (F ring; Z + warm ride it)
            #   gps [128,512]   bufs=1 -> 1 bank  (G ring)
            #   ups [128,256]   bufs=1 -> 1 bank  (U)
            def sps_tile():
                return ps.tile(
                    [128, N_CH, NF], f32, tag="sps", name="sps", bufs=2
                )

            def fps_tile():
                return ps.tile([128, NF], f32, tag="fps", name="fps", bufs=2)

            def gps_tile():
                return ps.tile([128, NF], f32, tag="gps", name="gps", bufs=1)

            def ups_tile():
                return ps.tile([128, C], f32, tag="ups", name="ups", bufs=1)

            # HAM warm-up: keep the PE busy until the first x chunk lands.
            warm_sb = const.tile([128, NF], bf16, tag="warm")
            nc.gpsimd.memset(warm_sb[:], 0.0)

            def warm(k):
                for _ in range(k):
                    wt = fps_tile()
                    nc.tensor.matmul(
                        wt[:], warm_sb[:, 0:128], warm_sb[:],
                        start=True, stop=True,
                    )

            warm(9)

            def load_x8(b):
                # fp8 x for all matmuls; n0 chunks first (G needs them first)
                x8 = xp.tile([128, CH_T, HW], fp8, tag="x8", name="x8")
                for n in range(N_CH):
                    for ci in range(CH_T):
                        nc.sync.dma_start(
                            out=x8[:, ci, ns(n)], in_=x8_d[b, mb(ci), ns(n)]
                        )
                return x8

            def load_xb(b):
                # bf16 x, only needed at finalize time
                xb = xbp.tile([128, CH_T, HW], bf16, tag="xb", name="xb")
                for ci in range(CH_T):
                    nc.sync.dma_start(out=xb[:, ci, :], in_=xb_d[b, mb(ci), :])
                return xb

            # ---- weights / constants ----
            mT8_sb = const.tile([128, CH_T, C], fp8, tag="mT8")
            nc.scalar.dma_start(
                out=mT8_sb[:], in_=mT8_d[:].rearrange("(t p) f -> p t f", p=128)
            )
            wu8_sb = const.tile([128, CH_T, C], fp8, tag="wu8")
            nc.scalar.dma_start(
                out=wu8_sb[:], in_=wu8_d[:].rearrange("(t p) f -> p t f", p=128)
            )
            bias_sb = const.tile([128, 4], f32, tag="bias")
            nc.scalar.dma_start(out=bias_sb[:], in_=bias_d[:])
            bf_sb = bias_sb[:, 0:2]
            wh_sb = bias_sb[:, 2:4]  # h = x^T Wk^T bq (zero for this model)
            # dual-fp8 ldweights ISA restriction (s3_lw.md): weights AP
            # must be [K, 2, M] with the k-pair dim step % 16 == 0 -> pad
            # the ones tile to stride 16 and slice M=4 columns
            ones8_t = const.tile([128, 2, 16], fp8, tag="ones8")
            nc.vector.memset(ones8_t[:], 1.0)
            ones8 = ones8_t[:, :, 0:4]
            nbias = const.tile([128, 1], f32, tag="nbias")
            nc.vector.memset(nbias[:], -ESHIFT)
            ones_row = const.tile([1, 128], bf16, tag="onesr")
            nc.vector.memset(ones_row[:], 1.0)

            x8_cur = load_x8(0)
            x8_next = load_x8(1) if BPC > 1 else None
            xb_cur = load_xb(0)

            def g_evict(g8, m, n, pst, eng):
                # GPSIMD cannot read PSUM: evictions go to DVE or ACT
                if not zero_bias:
                    nc.vector.tensor_scalar_add(
                        g8[:, m, ns(n)], pst[:], wh_sb[:, m : m + 1]
                    )
                elif eng == "act":
                    nc.scalar.copy(g8[:, m, ns(n)], pst[:])
                else:
                    nc.vector.tensor_copy(g8[:, m, ns(n)], pst[:])

            def g_mm(g8, m, n, x8, eng):
                pst = gps_tile()
                nc.tensor.matmul(
                    pst[:], mT8_sb[:, :, mb(m)], x8[:, :, ns(n)],
                    start=True, stop=True, perf_mode=DR,
                )
                g_evict(g8, m, n, pst, eng)

            def g_pass_head(x8):
                # batch 0 only: plain sequence before the window pipeline
                g8 = gp.tile([128, CH_T, HW], fp8, tag="g8", name="g8")
                for i, (n, m) in enumerate(
                    [(0, 0), (0, 1), (1, 0), (1, 1)]
                ):
                    g_mm(g8, m, n, x8, "act" if i % 2 else "dve")
                return g8

            def recip_bcast(st, zn, z_row):
                # 1/Z for one 512-wide half; after the second half, one
                # full-row DRAM round-trip broadcast across partitions
                nc.vector.reciprocal_approx_fast(
                    st["rrow"][:, ns(zn)], z_row
                )
                if zn < N_CH - 1:
                    return
                r_dram = rd.tile([1, HW], f32, tag="rdram")
                nc.sync.dma_start(out=r_dram[:], in_=st["rrow"][:])
                rb_sb = rp.tile([128, HW], f32, tag="rb", name="rb_sb")
                r_ap = r_dram[:]
                r_bc = bass.AP(
                    tensor=r_ap.tensor, offset=r_ap.offset,
                    ap=[[0, 128], [1, HW]],
                )
                nc.sync.dma_start(out=rb_sb[:], in_=r_bc)
                st["rb"][0] = rb_sb

            def norm_mul(f_psum, rb, mul_eng="vector"):
                # t = F*rb (PSUM fp32 x rb broadcast -> bf16); DVE only
                # (GPSIMD cannot read PSUM)
                t_sb = tp.tile([128, NF], bf16, tag="t", name="t_sb")
                nc.vector.tensor_mul(t_sb[:], f_psum, rb)
                return t_sb

            def add_store(b0, m, n, t_sb, xb, add_eng="gpsimd"):
                # o = t (+ b_f) + x; store bf16
                o_sb = op_.tile([128, NF], bf16, tag="o", name="o_sb")
                eng = nc.gpsimd if add_eng == "gpsimd" else nc.vector
                if zero_bias:
                    eng.tensor_add(o_sb[:], t_sb[:], xb[:, m, ns(n)])
                else:
                    nc.vector.scalar_tensor_tensor(
                        o_sb[:], t_sb[:], bf_sb[:, m : m + 1], xb[:, m, ns(n)],
                        op0=Alu.add, op1=Alu.add,
                    )
                nc.sync.dma_start(out=out_d[b0, mb(m), ns(n)], in_=o_sb[:])

            def norm_add_store(b0, m, n, f_psum, rb, xb, mul_eng="vector",
                               add_eng="gpsimd"):
                t_sb = norm_mul(f_psum, rb, mul_eng)
                add_store(b0, m, n, t_sb, xb, add_eng)

            prev = None  # state of batch b-1 {u8,e8,xb,b,zt,rb,fc,rrow}

            for b in range(BPC):
                x8, xb = x8_cur, xb_cur
                if b == 0:
                    g8 = g_pass_head(x8)
                    warm(4)  # bridge the g8-evict latency before S(m0)
                g8_next = (
                    gp.tile([128, CH_T, HW], fp8, tag="g8", name="g8")
                    if b + 1 < BPC
                    else None
                )
                if b + 1 < BPC:
                    xb_next = load_xb(b + 1)
                if b + 2 < BPC:
                    x8_fut = load_x8(b + 2)

                u8 = up.tile([128, HW_T, C], fp8, tag="u8", name="u8")
                e8 = ep.tile([128, HW_T, HW], fp8, tag="e8", name="e8")

                # F(b-1) DR schedule over steps 3..8 (counts sum to 16)
                FCNT = {3: 3, 4: 3, 5: 3, 6: 3, 7: 2, 8: 2}
                fdr = 0  # F DRs issued so far (4 per chunk)
                fins = []  # (chunk, t_sb) awaiting residual-add

                def f_issue(k):
                    nonlocal fdr
                    for _ in range(k):
                        ch, g = fdr // 4, fdr % 4
                        fm, fn = ch // N_CH, ch % N_CH
                        if g == 0:
                            prev["fc"][ch] = fps_tile()
                        nc.tensor.matmul(
                            prev["fc"][ch][:],
                            prev["u8"][:, 2 * g : 2 * g + 2, mb(fm)],
                            prev["e8"][:, 2 * g : 2 * g + 2, ns(fn)],
                            start=(g == 0), stop=(g == 3),
                            perf_mode=DR,
                        )
                        fdr += 1
                        if g == 3:
                            if ch < 2:
                                # mul now: frees the PSUM slot for c2/c3
                                t_sb = norm_mul(
                                    prev["fc"][ch][:],
                                    prev["rb"][0][:, ns(fn)],
                                )
                                fins.append((ch, t_sb))
                            else:
                                # c2/c3 muls deferred past Ue(m7) so the
                                # next window's U(m0) isn't queue-blocked
                                fins.append((ch, None))

                for m in range(HW_T):
                    lhsT = x8[:, :, mb(m)]
                    # S first: U's ups-ring wait (on Ue(m-?) deep in the
                    # DVE queue) must not head-block the PE stream
                    s2 = sps_tile()
                    for n in range(N_CH):
                        nc.tensor.matmul(
                            s2[:, n, :], lhsT, g8[:, :, ns(n)],
                            start=True, stop=True, perf_mode=DR,
                        )
                    pstU = ups_tile()
                    nc.tensor.matmul(
                        pstU[:], lhsT, wu8_sb[:],
                        start=True, stop=True, perf_mode=DR,
                    )
                    # --- evictions for this step (DVE; GPSIMD can't
                    # read PSUM) ---
                    nc.vector.tensor_copy(u8[:, m, :], pstU[:])
                    if b == BPC - 1 and m == HW_T - 1:
                        # last exp split per n-half: the epilogue Z n0
                        # chain starts half an exp earlier
                        for n in range(N_CH):
                            nc.scalar.activation(
                                e8[:, m, ns(n)], s2[:, n, :],
                                Act.Exp, scale=1.0 / math.sqrt(C),
                                bias=nbias[:],
                            )
                    else:
                        nc.scalar.activation(
                            e8[:, m, :], s2[:],
                            Act.Exp, scale=1.0 / math.sqrt(C), bias=nbias[:],
                        )
                    # --- interleaved deferred work for b-1 / b+1 ---
                    if prev is not None and m in (1, 2):
                        # Z(b-1) half m-1 -> fps slot rows [0:1]
                        zn = m - 1
                        zt = fps_tile()
                        prev["zt"][zn] = zt
                        for g in range(HW_T // 2):
                            nc.tensor.matmul(
                                zt[0:4, :], ones8,
                                prev["e8"][:, 2 * g : 2 * g + 2, ns(zn)],
                                start=(g == 0), stop=(g == HW_T // 2 - 1),
                                perf_mode=DR,
                            )
                        recip_bcast(prev, zn, zt[0:1, :])
                    if g8_next is not None and m in (2, 4, 6, 7):
                        gi = {2: 0, 4: 1, 6: 2, 7: 3}[m]
                        g_mm(
                            g8_next, gi % CH_T, gi // CH_T, x8_next,
                            "dve" if gi < 3 else "act",
                        )
                    if prev is not None and m >= 3:
                        f_issue(FCNT[m])
                    elif prev is None and m < 7:
                        warm(2)
                if prev is not None:
                    f_issue(FCNT[8])
                    # deferred c2/c3 muls, then batched adds + stores
                    fins2 = []
                    for ch, t_sb in fins:
                        fm, fn = ch // N_CH, ch % N_CH
                        if t_sb is None:
                            t_sb = norm_mul(
                                prev["fc"][ch][:],
                                prev["rb"][0][:, ns(fn)],
                            )
                        fins2.append((ch, t_sb))
                    for ch, t_sb in fins2:
                        fm, fn = ch // N_CH, ch % N_CH
                        add_store(prev["b"], fm, fn, t_sb, prev["xb"])

                prev = {
                    "b": b, "u8": u8, "e8": e8, "xb": xb,
                    "zt": [None, None], "rb": [None, None],
                    "fc": [None] * 4,
                    "rrow": rp.tile([1, HW], f32, tag="r", name="rrow"),
                }
                g8 = g8_next
                if b + 1 < BPC:
                    xb_cur = xb_next
                    x8_cur = x8_next
                if b + 2 < BPC:
                    x8_next = x8_fut

            # ---- epilogue: Z, F, finalize for the last batch ----
            bl = BPC - 1
            zt = [
                gps_tile(),
                ps.tile([128, NF], f32, tag="ups", name="zt1", bufs=1),
            ]
            for n in range(N_CH):
                for g in range(HW_T // 2):
                    nc.tensor.matmul(
                        zt[n][0:4, :], ones8,
                        prev["e8"][:, 2 * g : 2 * g + 2, ns(n)],
                        start=(g == 0), stop=(g == HW_T // 2 - 1),
                        perf_mode=DR,
                    )
            r_row = prev["rrow"]
            r16 = rp.tile([1, HW], bf16, tag="r16")
            rb_l = rp.tile([128, N_CH, NF], f32, tag="rbl")
            for n in range(N_CH):
                nc.vector.reciprocal_approx_fast(
                    r_row[:, ns(n)], zt[n][0:1, :]
                )
                nc.scalar.copy(r16[:, ns(n)], r_row[:, ns(n)])

            # F last batch: 4 chunks in the two sps tiles (scores all
            # consumed by now; nothing waits on the mul-gated fps ring)
            spsA, spsB = sps_tile(), sps_tile()
            fl = [spsA[:, 0, :], spsB[:, 0, :], spsA[:, 1, :],
                  spsB[:, 1, :]]

            def f_kloop(ch, pst):
                fm, fn = ch // N_CH, ch % N_CH
                for g in range(HW_T // 2):
                    nc.tensor.matmul(
                        pst, prev["u8"][:, 2 * g : 2 * g + 2, mb(fm)],
                        prev["e8"][:, 2 * g : 2 * g + 2, ns(fn)],
                        start=(g == 0), stop=(g == HW_T // 2 - 1),
                        perf_mode=DR,
                    )

            # n0 chunks first, rank-1 rb broadcast, then n1
            f_kloop(0, fl[0])  # m0 n0
            f_kloop(2, fl[2])  # m1 n0
            # rank-1 broadcast: rb_l[:, n, :] = ones x r16[:, ns(n)]
            rbp = [fps_tile(), fps_tile()]
            for n in range(N_CH):
                nc.tensor.matmul(
                    rbp[n][:], ones_row[:], r16[:, ns(n)],
                    start=True, stop=True,
                )
                nc.scalar.copy(rb_l[:, n, :], rbp[n][:])
            f_kloop(1, fl[1])  # m0 n1
            for ch in (0, 2):
                fm = ch // N_CH
                norm_add_store(
                    bl, fm, 0, fl[ch], rb_l[:, 0, :], prev["xb"],
                    add_eng="gpsimd" if ch == 0 else "vector",
                )
            f_kloop(3, fl[3])  # m1 n1
            norm_add_store(
                bl, 0, 1, fl[1], rb_l[:, 1, :], prev["xb"],
            )
            norm_add_store(
                bl, 1, 1, fl[3], rb_l[:, 1, :], prev["xb"],
                add_eng="vector",
            )

    nc.compile()
    return nc


def kernel(x, w_in, b_in, w_out, b_out):
    global last_results
    _ensure_path()
    import ml_dtypes
    from concourse import bass_utils

    trace = os.environ.get("ATTN_TRACE", "0") == "1"

    x = np.ascontiguousarray(np.asarray(x, dtype=np.float32))
    w_in = np.asarray(w_in, dtype=np.float32)
    b_in = np.asarray(b_in, dtype=np.float32)
    w_out = np.asarray(w_out, dtype=np.float32)
    b_out = np.asarray(b_out, dtype=np.float32)

    # host-side weight prep (tiny)
    w_q = w_in[:C]
    w_k = w_in[C : 2 * C]
    w_v = w_in[2 * C :]
    b_q = b_in[:C]
    b_v = b_in[2 * C :]
    w_u = w_out @ w_v  # fold output projection into V
    m_mat = w_k.T @ w_q  # [256, 256]
    w_h = w_k.T @ b_q  # h = x^T Wk^T bq, added during the G eviction
    b_f = w_out @ b_v + b_out  # [256]
    zero_bias = bool(np.all(w_h == 0.0) and np.all(b_f == 0.0))

    key = ("k", zero_bias)
    if key not in _cache:
        _cache[key] = _build(zero_bias)
    nc = _cache[key]

    f8 = ml_dtypes.float8_e4m3
    bfl = ml_dtypes.bfloat16
    mT8 = np.ascontiguousarray(m_mat.T.astype(f8))
    wu8 = np.ascontiguousarray(w_u.T.astype(f8))  # [256, 256]
    bias = np.stack(
        [b_f[:128], b_f[128:], w_h[:128], w_h[128:]], axis=1
    )  # [128, 4]
    bias = np.ascontiguousarray(bias.astype(np.float32))

    xr = x.reshape(B, C, HW)
    x8 = xr.astype(f8)
    xb = xr.astype(bfl)
    in_maps = []
    for c in range(NCORES):
        m = {
            "x8": np.ascontiguousarray(x8[c * BPC : (c + 1) * BPC]),
            "xb": np.ascontiguousarray(xb[c * BPC : (c + 1) * BPC]),
            "mT8": mT8,
            "wu8": wu8,
            "bias": bias,
        }
        in_maps.append(m)

    res = bass_utils.run_bass_kernel_spmd(
        nc, in_maps, core_ids=list(range(NCORES)), trace=trace
    )
    last_results = res

    out = np.concatenate([res.results[i]["out"] for i in range(NCORES)], axis=0)
    return out.reshape(B, C, 32, 32).astype(np.float32)
